# revision 5
# baseline (speedup 1.0000x reference)
"""Trainium2 Bass kernel v2 for GeneralNonLinearReadoutBlock (gated equivariant MLP).

Strategy vs v1: all device traffic in bf16 (halves HBM bytes — the kernel is
memory-bound), and the host ships x already transposed + de-interleaved so the
device does zero transposes:

  host:  x [N,512] fp32  ->  xT [128(c), 4(group), rows] bf16 per core
         groups: 0 = scalars x0;  1+i = vector component i (channel-major)
         weights pre-scaled by 1/sqrt(128), cast bf16
  device per 1792-row chunk (14 subtiles of 128 nodes):
         one DMA in; per macrotile (nf<=512 nodes):
           mm1 (weights stationary, feature-major out)  -> PSUM
           silu gates on ACT, vector gating muls on DVE -> bf16 SBUF
           mm2 (activations stationary -> node-major out) -> PSUM
           PSUM->SBUF copies into yout (ACT/DVE)
         one DMA out:  y [rows, 512] bf16, col-groups [y0|y1_0|y1_1|y1_2]
  host:  re-interleave y columns, cast fp32
"""

import sys

sys.path.insert(0, "/opt/trn_rl_repo")

import numpy as np
import ml_dtypes

import concourse.bass as bass
import concourse.tile as tile
from concourse import mybir
from concourse._compat import not_none as nn
from concourse.vector_clock import ScopedClock

MUL = 128
N_FULL = 100000
N_CORES = 8
ROWS_PER_CORE = 12544  # 98 subtiles of 128; 8*12544 = 100352 (pad 352 rows)
F = 4 * MUL  # 512 features
INV = np.float32(1.0 / np.sqrt(np.float32(MUL)))
BF16 = mybir.dt.bfloat16
FP32 = mybir.dt.float32

import os as _os

CHUNK_T = int(_os.environ.get("KOPT_CHUNK_T", "14"))   # subtiles per chunk
MACRO_S = int(_os.environ.get("KOPT_MACRO_S", "4"))    # subtiles per macrotile
XIN_BUFS = int(_os.environ.get("KOPT_XIN_BUFS", "3"))
YOUT_BUFS = int(_os.environ.get("KOPT_YOUT_BUFS", "2"))


class SplitDrainTileContext(tile.TileContext):
    """TileContext whose final drain splits sem waits across SP nops.

    The pinned walrus rejects >1 sync-wait on a TPB_CTRL drain; stock
    TileContext puts every outstanding proc's wait on the one tail drain.
    """

    MAXW = 1

    def _split_waits_everywhere(self):
        nc = self.nc
        cur = nn(nc.cur_bb).bb
        eng_map = {
            mybir.EngineType.PE: nc.tensor,
            mybir.EngineType.DVE: nc.vector,
            mybir.EngineType.Activation: nc.scalar,
            mybir.EngineType.Pool: nc.gpsimd,
            mybir.EngineType.SP: nc.sync,
        }
        for f in nc.m.functions:
            for bb in f.blocks:
                new_insts = []
                changed = False
                for inst in bb.instructions:
                    si = inst.sync_info
                    waits = list(si.on_wait) if si is not None else []
                    if len(waits) > self.MAXW:
                        changed = True
                        chunks = [
                            waits[i : i + self.MAXW]
                            for i in range(0, len(waits), self.MAXW)
                        ]
                        for chunk in chunks[:-1]:
                            nop = eng_map[inst.engine].nop(
                                nofuse=True, hint="wait_split"
                            )
                            assert cur.instructions[-1] is nop.ins
                            cur.instructions.pop()
                            nop.ins.sync_info = mybir.SyncInfo(
                                on_wait=chunk, on_update=[]
                            )
                            new_insts.append(nop.ins)
                        si.on_wait = chunks[-1]
                        inst.sync_info = si
                    new_insts.append(inst)
                if changed:
                    bb.instructions[:] = new_insts

    def _drain_and_barrier(self, tick_clock, wait_clock):
        self._split_waits_everywhere()
        drain_inst = self.nc.sync.drain()
        wait_clock.add_sem_waits(
            drain_inst.ins, ScopedClock({None: tick_clock.global_clock})
        )
        waits = list(drain_inst.ins.sync_info.on_wait)
        if len(waits) > self.MAXW:
            chunks = [waits[i : i + self.MAXW] for i in range(0, len(waits), self.MAXW)]
            si = drain_inst.ins.sync_info
            si.on_wait = chunks[-1]
            drain_inst.ins.sync_info = si
            bb = nn(self.nc.cur_bb).bb
            assert bb.instructions[-1] is drain_inst.ins
            bb.instructions.pop()
            for chunk in chunks[:-1]:
                nop = self.nc.sync.nop(nofuse=True, hint="drain_wait_split")
                nop.ins.sync_info = mybir.SyncInfo(on_wait=chunk, on_update=[])
            bb.instructions.append(drain_inst.ins)
        self.nc.all_engine_barrier()
        assert self.sems is not None
        popped = self.nc._tile_sem_poison_stack.pop()
        assert popped is self._sem_poison
        self.nc.clear_and_free_semaphores(list(self.sems.allocated().values()))
        self.nc.all_engine_barrier()


DEFAULT_CFG = dict(
    chunk_t=7, macro_s=4, xin_bufs=4, yout_bufs=3,
    mul_eng="vvv", cpy_eng="aavv", ps_s=2, ps_v=2, ps_y=4,
    h0_bufs=3, g_bufs=3, h1_bufs=6, out_split=1,
    fuse_s=False, fuse_v=False, fuse_y=False, mm2_wstat=True, taper=False,
    xblock=True, packw=True,
    silu_eng="a", fmul_eng="v", fcpy_eng="v", ycpy0_eng="a",
    out_dma_eng="a",
)


def build_ir(tc, y_d, x_d, w1s_d, w1v_d, w2s_d, w2v_d, n_rows, repeats=1, cfg=None):
    cfg = {**DEFAULT_CFG, **(cfg or {})}
    ENG = None
    nc = tc.nc
    ENG = {"a": nc.scalar, "v": nc.vector, "p": nc.gpsimd}
    assert n_rows % 128 == 0
    n_tiles = n_rows // 128
    CT = cfg["chunk_t"]
    chunks = [CT] * (n_tiles // CT)
    if n_tiles % CT:
        chunks.append(n_tiles % CT)
    if cfg["taper"] and chunks and chunks[-1] > 2:
        T = chunks.pop()
        while T > 1:
            t = (T + 1) // 2
            chunks.append(t)
            T -= t
        chunks.append(1)

    with (
        tc.tile_pool(name="consts", bufs=1) as consts,
        tc.tile_pool(name="xin", bufs=cfg["xin_bufs"]) as xin_pool,
        tc.tile_pool(name="h0", bufs=cfg["h0_bufs"]) as h0_pool,
        tc.tile_pool(name="g", bufs=cfg["g_bufs"]) as g_pool,
        tc.tile_pool(name="h1", bufs=cfg["h1_bufs"]) as h1_pool,
        tc.tile_pool(name="yout", bufs=cfg["yout_bufs"]) as yout_pool,
        tc.tile_pool(name="psS", bufs=cfg["ps_s"], space="PSUM") as psS,
        tc.tile_pool(name="psV", bufs=cfg["ps_v"], space="PSUM") as psV,
        tc.tile_pool(name="psY", bufs=cfg["ps_y"], space="PSUM") as psY,
    ):
        # ---- weights (already bf16 + pre-scaled host-side) ---------------
        if cfg["packw"]:
            wall = consts.tile([128, 5 * MUL], BF16)
            nc.sync.dma_start(wall[:], w1s_d[:, :])  # w1s_d is the packed [128,640]
            w1s = wall[:, 0 : 2 * MUL]
            w1v = wall[:, 2 * MUL : 3 * MUL]
            w2s = wall[:, 3 * MUL : 4 * MUL]
            w2v = wall[:, 4 * MUL : 5 * MUL]
        else:
            w1s_t = consts.tile([128, 2 * MUL], BF16)
            w1v_t = consts.tile([128, MUL], BF16)
            w2s_t = consts.tile([128, MUL], BF16)
            w2v_t = consts.tile([128, MUL], BF16)
            nc.sync.dma_start(w1s_t[:], w1s_d[:, :])
            nc.sync.dma_start(w1v_t[:], w1v_d[:, :])
            nc.sync.dma_start(w2s_t[:], w2s_d[:, :])
            nc.sync.dma_start(w2v_t[:], w2v_d[:, :])
            w1s, w1v, w2s, w2v = w1s_t[:], w1v_t[:], w2s_t[:], w2v_t[:]

        for _rep in range(repeats):
            t0 = 0  # subtile cursor within the core
            for ci, T in enumerate(chunks):
                rows = T * 128
                r0 = t0 * 128

                # ---- load chunk: [128(c), 4(g), rows] bf16 ---------------
                xin = xin_pool.tile([128, 4, rows], BF16, tag="xin")
                if cfg["xblock"]:
                    nc.sync.dma_start(xin[:], x_d[:, ci, :, :])
                else:
                    nc.sync.dma_start(xin[:], x_d[:, :, r0 : r0 + rows])

                if cfg["mm2_wstat"]:
                    yout = yout_pool.tile([128, 4, rows], BF16, tag="yout")
                else:
                    yout = yout_pool.tile([128, T, F], BF16, tag="yout")

                # ---- macrotiles ------------------------------------------
                s0 = 0
                while s0 < T:
                    S = min(cfg["macro_s"], T - s0)
                    nf = S * 128
                    sl = slice(s0 * 128, s0 * 128 + nf)

                    # ---- mm1: feature-major out [c_out, n] -----------
                    if cfg["fuse_s"]:
                        ps_s = psS.tile([128, 2, nf], FP32, tag="psS")
                        nc.tensor.matmul(
                            ps_s[:, 0, :], w1s[:, 0:MUL], xin[:, 0, sl],
                            start=True, stop=True,
                        )
                        nc.tensor.matmul(
                            ps_s[:, 1, :], w1s[:, MUL:], xin[:, 0, sl],
                            start=True, stop=True,
                        )
                    else:
                        ps_a = psS.tile([128, nf], FP32, tag="psS")
                        nc.tensor.matmul(
                            ps_a[:], w1s[:, 0:MUL], xin[:, 0, sl], start=True, stop=True
                        )
                        ps_b = psS.tile([128, nf], FP32, tag="psS")
                        nc.tensor.matmul(
                            ps_b[:], w1s[:, MUL:], xin[:, 0, sl], start=True, stop=True
                        )
                    if cfg["fuse_v"]:
                        ps_vt = psV.tile([128, 3, nf], FP32, tag="psV")
                        for i in range(3):
                            nc.tensor.matmul(
                                ps_vt[:, i, :], w1v[:], xin[:, 1 + i, sl],
                                start=True, stop=True,
                            )
                    else:
                        ps_v = []
                        for i in range(3):
                            pv = psV.tile([128, nf], FP32, tag="psV")
                            nc.tensor.matmul(
                                pv[:], w1v, xin[:, 1 + i, sl], start=True, stop=True
                            )
                            ps_v.append(pv)

                    # ---- gate ----------------------------------------
                    SILU = mybir.ActivationFunctionType.Silu
                    if cfg["fuse_s"]:
                        h0g = h0_pool.tile([128, 2, nf], BF16, tag="h0")
                        nc.scalar.activation(h0g[:], ps_s[:], SILU)
                        h0 = h0g[:, 0, :]
                        g_ap = h0g[:, 1, :]
                    else:
                        h0t = h0_pool.tile([128, nf], BF16, tag="h0")
                        nc.scalar.activation(h0t[:], ps_a[:], SILU)
                        h0 = h0t[:]
                        gt = g_pool.tile([128, nf], FP32, tag="g")
                        nc.scalar.activation(gt[:], ps_b[:], SILU)
                        g_ap = gt[:]
                    if cfg["fuse_v"]:
                        h1t = h1_pool.tile([128, 3, nf], BF16, tag="h1")
                        gb = g_ap.rearrange("p (o n) -> p o n", o=1).broadcast_to(
                            [128, 3, nf]
                        )
                        ENG[cfg["fmul_eng"]].tensor_mul(h1t[:], ps_vt[:], gb)
                        h1 = [h1t[:, i, :] for i in range(3)]
                    else:
                        h1 = []
                        for i in range(3):
                            hi = h1_pool.tile([128, nf], BF16, tag="h1")
                            ENG[cfg["mul_eng"][i]].tensor_mul(hi[:], ps_v[i][:], g_ap)
                            h1.append(hi[:])

                    # ---- mm2: activations stationary -> node-major ----
                    def _copy(eng_key, dst, src_ap):
                        eng = ENG[eng_key]
                        if eng is nc.scalar:
                            eng.copy(dst, src_ap)
                        else:
                            eng.tensor_copy(dst, src_ap)

                    if cfg["mm2_wstat"]:
                        for pidx, (act, w2) in enumerate(
                            [(h0, w2s), (h1[0], w2v), (h1[1], w2v), (h1[2], w2v)]
                        ):
                            py = psY.tile([128, nf], FP32, tag="psY")
                            nc.tensor.matmul(
                                py[:], w2, act[:], start=True, stop=True
                            )
                            dst = yout[:, pidx, sl]
                            _copy(cfg["cpy_eng"][pidx], dst, py[:])
                    elif cfg["fuse_y"]:
                        py0 = psY.tile([128, nf], FP32, tag="psY0")
                        for j in range(S):
                            nc.tensor.matmul(
                                py0[:, j * 128 : (j + 1) * 128],
                                h0[:, j * 128 : (j + 1) * 128],
                                w2s, start=True, stop=True,
                            )
                        _copy(
                            cfg["ycpy0_eng"],
                            yout[:, s0 : s0 + S, 0:128],
                            py0[:].rearrange("p (s n) -> p s n", s=S),
                        )
                        pyv = psY.tile([128, 3, nf], FP32, tag="psYv")
                        for i in range(3):
                            for j in range(S):
                                nc.tensor.matmul(
                                    pyv[:, i, j * 128 : (j + 1) * 128],
                                    h1[i][:, j * 128 : (j + 1) * 128],
                                    w2v, start=True, stop=True,
                                )
                        dst = yout[:, s0 : s0 + S, MUL:].rearrange(
                            "p s (c n) -> p s c n", c=3
                        )
                        src_ap = pyv[:].rearrange("p c (s n) -> p s c n", s=S)
                        _copy(cfg["fcpy_eng"], dst, src_ap)
                    else:
                        for pidx, (act, w2) in enumerate(
                            [(h0, w2s), (h1[0], w2v), (h1[1], w2v), (h1[2], w2v)]
                        ):
                            py = psY.tile([128, nf], FP32, tag="psY")
                            for j in range(S):
                                nc.tensor.matmul(
                                    py[:, j * 128 : (j + 1) * 128],
                                    act[:, j * 128 : (j + 1) * 128],
                                    w2, start=True, stop=True,
                                )
                            dst = yout[:, s0 : s0 + S, pidx * 128 : (pidx + 1) * 128]
                            src_ap = py[:].rearrange("p (s n) -> p s n", s=S)
                            _copy(cfg["cpy_eng"][pidx], dst, src_ap)

                    s0 += S

                # ---- store chunk (optionally in pieces for earlier drain) -
                out_eng = nc.sync if cfg["out_dma_eng"] == "s" else nc.scalar
                if cfg["mm2_wstat"]:
                    if cfg["xblock"]:
                        out_eng.dma_start(y_d[:, ci, :, :], yout[:])
                    else:
                        out_eng.dma_start(y_d[:, :, r0 : r0 + rows], yout[:])
                else:
                    n_out = cfg.get("out_split", 1)
                    tper = (T + n_out - 1) // n_out
                    o0 = 0
                    while o0 < T:
                        ot = min(tper, T - o0)
                        dst = y_d[r0 + o0 * 128 : r0 + (o0 + ot) * 128, :].rearrange(
                            "(t p) f -> p t f", p=128
                        )
                        out_eng.dma_start(dst, yout[:, o0 : o0 + ot, :])
                        o0 += ot

                t0 += T


def build_bass(n_rows=ROWS_PER_CORE, repeats=1, cfg=None):
    full_cfg = {**DEFAULT_CFG, **(cfg or {})}
    nc = bass.Bass(trn_type="TRN2", target_bir_lowering=False, debug=False)
    if full_cfg["xblock"]:
        ct = full_cfg["chunk_t"]
        assert (n_rows // 128) % ct == 0 and not full_cfg["taper"]
        nch = (n_rows // 128) // ct
        x_d = nc.dram_tensor(
            "x", [128, nch, 4, ct * 128], BF16, kind="ExternalInput"
        ).ap()
    else:
        x_d = nc.dram_tensor("x", [128, 4, n_rows], BF16, kind="ExternalInput").ap()
    if full_cfg["packw"]:
        w1s_d = nc.dram_tensor("w", [MUL, 5 * MUL], BF16, kind="ExternalInput").ap()
        w1v_d = w2s_d = w2v_d = None
    else:
        w1s_d = nc.dram_tensor("w1_s", [MUL, 2 * MUL], BF16, kind="ExternalInput").ap()
        w1v_d = nc.dram_tensor("w1_v", [MUL, MUL], BF16, kind="ExternalInput").ap()
        w2s_d = nc.dram_tensor("w2_s", [MUL, MUL], BF16, kind="ExternalInput").ap()
        w2v_d = nc.dram_tensor("w2_v", [MUL, MUL], BF16, kind="ExternalInput").ap()
    if full_cfg["mm2_wstat"]:
        if full_cfg["xblock"]:
            y_d = nc.dram_tensor(
                "y", [128, nch, 4, ct * 128], BF16, kind="ExternalOutput"
            ).ap()
        else:
            y_d = nc.dram_tensor(
                "y", [128, 4, n_rows], BF16, kind="ExternalOutput"
            ).ap()
    else:
        y_d = nc.dram_tensor("y", [n_rows, F], BF16, kind="ExternalOutput").ap()
    with SplitDrainTileContext(nc) as tc:
        build_ir(tc, y_d, x_d, w1s_d, w1v_d, w2s_d, w2v_d, n_rows, repeats=repeats, cfg=cfg)
    return nc


def _pack_x(x_shard, cfg=None):
    """[rows, 512] fp32 -> [128(c), 4(g), rows] bf16, de-interleaved.

    With cfg[xblock]: -> [128, n_chunks, 4, chunk_rows] (chunk-contiguous)."""
    cfg = {**DEFAULT_CFG, **(cfg or {})}
    rows = x_shard.shape[0]
    xb = x_shard.astype(ml_dtypes.bfloat16)
    out = np.empty((128, 4, rows), ml_dtypes.bfloat16)
    out[:, 0, :] = xb[:, :MUL].T
    x1 = xb[:, MUL:].reshape(rows, MUL, 3)
    for i in range(3):
        out[:, 1 + i, :] = x1[:, :, i].T
    if cfg["xblock"]:
        ctr = cfg["chunk_t"] * 128
        nch = rows // ctr
        # [128, 4, nch, ctr] -> [128, nch, 4, ctr]
        out = np.ascontiguousarray(
            out.reshape(128, 4, nch, ctr).transpose(0, 2, 1, 3)
        )
    return out


def shard_inputs(x, w1_s, w1_v, w2_s, w2_v, n_rows=ROWS_PER_CORE, cfg=None):
    x = np.ascontiguousarray(np.asarray(x, dtype=np.float32))
    pad = N_CORES * n_rows - x.shape[0]
    if pad:
        x = np.concatenate([x, np.zeros((pad, x.shape[1]), np.float32)], axis=0)
    shards = x.reshape(N_CORES, n_rows, F)
    full_cfg = {**DEFAULT_CFG, **(cfg or {})}
    if full_cfg["packw"]:
        wcat = np.concatenate(
            [
                np.asarray(w1_s, np.float32),
                np.asarray(w1_v, np.float32),
                np.asarray(w2_s, np.float32),
                np.asarray(w2_v, np.float32),
            ],
            axis=1,
        )
        w = {"w": (wcat * INV).astype(ml_dtypes.bfloat16)}
    else:
        w = {
            "w1_s": (np.asarray(w1_s, np.float32) * INV).astype(ml_dtypes.bfloat16),
            "w1_v": (np.asarray(w1_v, np.float32) * INV).astype(ml_dtypes.bfloat16),
            "w2_s": (np.asarray(w2_s, np.float32) * INV).astype(ml_dtypes.bfloat16),
            "w2_v": (np.asarray(w2_v, np.float32) * INV).astype(ml_dtypes.bfloat16),
        }
    return [dict(w, x=_pack_x(shards[c], cfg=cfg)) for c in range(N_CORES)]


def _unpack_y(y_all, n_out):
    """[n, 512] bf16 col-groups [y0|y1_0|y1_1|y1_2] -> fp32 interleaved."""
    y = y_all[:n_out]
    out = np.empty((n_out, F), np.float32)
    out[:, :MUL] = y[:, :MUL]
    yv = np.asarray(y[:, MUL:], dtype=np.float32).reshape(n_out, 3, MUL)
    out[:, MUL:] = yv.transpose(0, 2, 1).reshape(n_out, 3 * MUL)
    return out


def _unpack_y_wstat(y_cores, n_out):
    """list of [128(v), 4(g), rows] (or chunk-blocked) bf16 -> fp32 interleaved."""
    out_blocks = []
    for yd in y_cores:
        if yd.ndim == 4:  # [128, nch, 4, ctr] -> [128, 4, rows]
            yd = yd.transpose(0, 2, 1, 3).reshape(128, 4, -1)
        rows = yd.shape[2]
        blk = np.empty((rows, F), np.float32)
        blk[:, :MUL] = yd[:, 0, :].T
        yv = np.asarray(yd[:, 1:, :], dtype=np.float32)  # [128v, 3, rows]
        blk[:, MUL:] = yv.transpose(2, 0, 1).reshape(rows, 3 * MUL)
        out_blocks.append(blk)
    return np.concatenate(out_blocks, axis=0)[:n_out]


_NC_CACHE = {}


def kernel(x, w1_s, w1_v, w2_s, w2_v):
    from concourse.bass_utils import run_bass_kernel_spmd

    if "nc" not in _NC_CACHE:
        _NC_CACHE["nc"] = build_bass()
    nc = _NC_CACHE["nc"]
    in_maps = shard_inputs(x, w1_s, w1_v, w2_s, w2_v)
    res = run_bass_kernel_spmd(nc, in_maps, core_ids=list(range(N_CORES)))
    if DEFAULT_CFG["mm2_wstat"]:
        return _unpack_y_wstat([res.results[c]["y"] for c in range(N_CORES)], N_FULL)
    y = np.concatenate([res.results[c]["y"] for c in range(N_CORES)], axis=0)
    return _unpack_y(y, N_FULL)


# revision 6
# speedup vs baseline: 1.1492x; 1.1492x over previous
"""Trainium2 Bass kernel for GeneralNonLinearReadoutBlock (gated equivariant MLP).

Reference (per node): x -> linear1 -> silu gate -> linear2, channels MUL=128,
vectors interleaved as (u, xyz). The kernel is memory-bound (headroom target),
so the design minimizes HBM bytes and device-side data motion:

  host (inside kernel(), numpy only):
    - x [N,512] fp32 -> bf16, de-interleaved into 4 channel groups
      (scalars | vec_x | vec_y | vec_z), TRANSPOSED to feature-major, and
      chunk-blocked: x_dev [128(c), n_chunks, 4(group), 896] per core, so each
      chunk DMA is one fully contiguous 7KB run per partition.
    - all 4 weight mats pre-scaled by 1/sqrt(128), bf16, packed into one
      [128, 640] tensor (one DMA, fewer per-call args).
  device (per 896-row chunk, macrotiles of nf<=512 nodes):
    - mm1 weights-stationary (bf16, 1 cyc/row): feature-major s/gates/v PSUM
    - silu on ACT -> bf16 h0 + fp32 g; gating muls on DVE -> bf16 h1
    - mm2 weights-stationary too: y^T feature-major PSUM (4 matmuls/macro)
    - ACT/DVE copies PSUM -> chunk-blocked y_dev [128, n_chunks, 4, 896] bf16
  host: un-transpose + re-interleave y, cast fp32.

No transposes on device at all; both DMA directions move bf16 at max
descriptor contiguity. Measured ~93us/core/pass (repeats-slope) vs 139us for
the fp32 baseline; bf16 end-to-end rel err 6.3e-3 vs the 2e-2 gate.

Engine/PSUM config chosen by timeline-sim sweep (sweep_v2.py): PSUM banks
2/2/4 (mm1-s / mm1-v / mm2), gate muls on DVE, copies alternating ACT/DVE.
GPSIMD is unusable here: the BIR verifier forbids GPSIMD<->PSUM access.
"""

import sys

sys.path.insert(0, "/opt/trn_rl_repo")

import numpy as np
import ml_dtypes

import concourse.bass as bass
import concourse.tile as tile
from concourse import mybir
from concourse._compat import not_none as nn
from concourse.vector_clock import ScopedClock

MUL = 128
N_FULL = 100000
N_CORES = 8
ROWS_PER_CORE = 12544  # 98 subtiles of 128; 8*12544 = 100352 (pad 352 rows)
F = 4 * MUL  # 512 features
INV = np.float32(1.0 / np.sqrt(np.float32(MUL)))
BF16 = mybir.dt.bfloat16
FP32 = mybir.dt.float32

import os as _os

CHUNK_T = int(_os.environ.get("KOPT_CHUNK_T", "14"))   # subtiles per chunk
MACRO_S = int(_os.environ.get("KOPT_MACRO_S", "4"))    # subtiles per macrotile
XIN_BUFS = int(_os.environ.get("KOPT_XIN_BUFS", "3"))
YOUT_BUFS = int(_os.environ.get("KOPT_YOUT_BUFS", "2"))


class SplitDrainTileContext(tile.TileContext):
    """TileContext whose final drain splits sem waits across SP nops.

    The pinned walrus rejects >1 sync-wait on a TPB_CTRL drain; stock
    TileContext puts every outstanding proc's wait on the one tail drain.
    """

    MAXW = 1

    def _split_waits_everywhere(self):
        nc = self.nc
        cur = nn(nc.cur_bb).bb
        eng_map = {
            mybir.EngineType.PE: nc.tensor,
            mybir.EngineType.DVE: nc.vector,
            mybir.EngineType.Activation: nc.scalar,
            mybir.EngineType.Pool: nc.gpsimd,
            mybir.EngineType.SP: nc.sync,
        }
        for f in nc.m.functions:
            for bb in f.blocks:
                new_insts = []
                changed = False
                for inst in bb.instructions:
                    si = inst.sync_info
                    waits = list(si.on_wait) if si is not None else []
                    if len(waits) > self.MAXW:
                        changed = True
                        chunks = [
                            waits[i : i + self.MAXW]
                            for i in range(0, len(waits), self.MAXW)
                        ]
                        for chunk in chunks[:-1]:
                            nop = eng_map[inst.engine].nop(
                                nofuse=True, hint="wait_split"
                            )
                            assert cur.instructions[-1] is nop.ins
                            cur.instructions.pop()
                            nop.ins.sync_info = mybir.SyncInfo(
                                on_wait=chunk, on_update=[]
                            )
                            new_insts.append(nop.ins)
                        si.on_wait = chunks[-1]
                        inst.sync_info = si
                    new_insts.append(inst)
                if changed:
                    bb.instructions[:] = new_insts

    def _drain_and_barrier(self, tick_clock, wait_clock):
        self._split_waits_everywhere()
        drain_inst = self.nc.sync.drain()
        wait_clock.add_sem_waits(
            drain_inst.ins, ScopedClock({None: tick_clock.global_clock})
        )
        waits = list(drain_inst.ins.sync_info.on_wait)
        if len(waits) > self.MAXW:
            chunks = [waits[i : i + self.MAXW] for i in range(0, len(waits), self.MAXW)]
            si = drain_inst.ins.sync_info
            si.on_wait = chunks[-1]
            drain_inst.ins.sync_info = si
            bb = nn(self.nc.cur_bb).bb
            assert bb.instructions[-1] is drain_inst.ins
            bb.instructions.pop()
            for chunk in chunks[:-1]:
                nop = self.nc.sync.nop(nofuse=True, hint="drain_wait_split")
                nop.ins.sync_info = mybir.SyncInfo(on_wait=chunk, on_update=[])
            bb.instructions.append(drain_inst.ins)
        self.nc.all_engine_barrier()
        assert self.sems is not None
        popped = self.nc._tile_sem_poison_stack.pop()
        assert popped is self._sem_poison
        self.nc.clear_and_free_semaphores(list(self.sems.allocated().values()))
        self.nc.all_engine_barrier()


DEFAULT_CFG = dict(
    chunk_t=7, macro_s=4, xin_bufs=4, yout_bufs=3,
    mul_eng="vvv", cpy_eng="aavv", ps_s=2, ps_v=2, ps_y=4,
    h0_bufs=3, g_bufs=3, h1_bufs=6, out_split=1,
    fuse_s=False, fuse_v=False, fuse_y=False, mm2_wstat=True, taper=False,
    xblock=True, packw=True,
    silu_eng="a", fmul_eng="v", fcpy_eng="v", ycpy0_eng="a",
    out_dma_eng="a",
)


def build_ir(tc, y_d, x_d, w1s_d, w1v_d, w2s_d, w2v_d, n_rows, repeats=1, cfg=None):
    cfg = {**DEFAULT_CFG, **(cfg or {})}
    ENG = None
    nc = tc.nc
    ENG = {"a": nc.scalar, "v": nc.vector, "p": nc.gpsimd}
    assert n_rows % 128 == 0
    n_tiles = n_rows // 128
    CT = cfg["chunk_t"]
    chunks = [CT] * (n_tiles // CT)
    if n_tiles % CT:
        chunks.append(n_tiles % CT)
    if cfg["taper"] and chunks and chunks[-1] > 2:
        T = chunks.pop()
        while T > 1:
            t = (T + 1) // 2
            chunks.append(t)
            T -= t
        chunks.append(1)

    with (
        tc.tile_pool(name="consts", bufs=1) as consts,
        tc.tile_pool(name="xin", bufs=cfg["xin_bufs"]) as xin_pool,
        tc.tile_pool(name="h0", bufs=cfg["h0_bufs"]) as h0_pool,
        tc.tile_pool(name="g", bufs=cfg["g_bufs"]) as g_pool,
        tc.tile_pool(name="h1", bufs=cfg["h1_bufs"]) as h1_pool,
        tc.tile_pool(name="yout", bufs=cfg["yout_bufs"]) as yout_pool,
        tc.tile_pool(name="psS", bufs=cfg["ps_s"], space="PSUM") as psS,
        tc.tile_pool(name="psV", bufs=cfg["ps_v"], space="PSUM") as psV,
        tc.tile_pool(name="psY", bufs=cfg["ps_y"], space="PSUM") as psY,
    ):
        # ---- weights (already bf16 + pre-scaled host-side) ---------------
        if cfg["packw"]:
            wall = consts.tile([128, 5 * MUL], BF16)
            nc.sync.dma_start(wall[:], w1s_d[:, :])  # w1s_d is the packed [128,640]
            w1s = wall[:, 0 : 2 * MUL]
            w1v = wall[:, 2 * MUL : 3 * MUL]
            w2s = wall[:, 3 * MUL : 4 * MUL]
            w2v = wall[:, 4 * MUL : 5 * MUL]
        else:
            w1s_t = consts.tile([128, 2 * MUL], BF16)
            w1v_t = consts.tile([128, MUL], BF16)
            w2s_t = consts.tile([128, MUL], BF16)
            w2v_t = consts.tile([128, MUL], BF16)
            nc.sync.dma_start(w1s_t[:], w1s_d[:, :])
            nc.sync.dma_start(w1v_t[:], w1v_d[:, :])
            nc.sync.dma_start(w2s_t[:], w2s_d[:, :])
            nc.sync.dma_start(w2v_t[:], w2v_d[:, :])
            w1s, w1v, w2s, w2v = w1s_t[:], w1v_t[:], w2s_t[:], w2v_t[:]

        for _rep in range(repeats):
            t0 = 0  # subtile cursor within the core
            for ci, T in enumerate(chunks):
                rows = T * 128
                r0 = t0 * 128

                # ---- load chunk: [128(c), 4(g), rows] bf16 ---------------
                xin = xin_pool.tile([128, 4, rows], BF16, tag="xin")
                if cfg["xblock"]:
                    nc.sync.dma_start(xin[:], x_d[:, ci, :, :])
                else:
                    nc.sync.dma_start(xin[:], x_d[:, :, r0 : r0 + rows])

                if cfg["mm2_wstat"]:
                    yout = yout_pool.tile([128, 4, rows], BF16, tag="yout")
                else:
                    yout = yout_pool.tile([128, T, F], BF16, tag="yout")

                # ---- macrotiles ------------------------------------------
                s0 = 0
                while s0 < T:
                    S = min(cfg["macro_s"], T - s0)
                    nf = S * 128
                    sl = slice(s0 * 128, s0 * 128 + nf)

                    # ---- mm1: feature-major out [c_out, n] -----------
                    if cfg["fuse_s"]:
                        ps_s = psS.tile([128, 2, nf], FP32, tag="psS")
                        nc.tensor.matmul(
                            ps_s[:, 0, :], w1s[:, 0:MUL], xin[:, 0, sl],
                            start=True, stop=True,
                        )
                        nc.tensor.matmul(
                            ps_s[:, 1, :], w1s[:, MUL:], xin[:, 0, sl],
                            start=True, stop=True,
                        )
                    else:
                        ps_a = psS.tile([128, nf], FP32, tag="psS")
                        nc.tensor.matmul(
                            ps_a[:], w1s[:, 0:MUL], xin[:, 0, sl], start=True, stop=True
                        )
                        ps_b = psS.tile([128, nf], FP32, tag="psS")
                        nc.tensor.matmul(
                            ps_b[:], w1s[:, MUL:], xin[:, 0, sl], start=True, stop=True
                        )
                    if cfg["fuse_v"]:
                        ps_vt = psV.tile([128, 3, nf], FP32, tag="psV")
                        for i in range(3):
                            nc.tensor.matmul(
                                ps_vt[:, i, :], w1v[:], xin[:, 1 + i, sl],
                                start=True, stop=True,
                            )
                    else:
                        ps_v = []
                        for i in range(3):
                            pv = psV.tile([128, nf], FP32, tag="psV")
                            nc.tensor.matmul(
                                pv[:], w1v, xin[:, 1 + i, sl], start=True, stop=True
                            )
                            ps_v.append(pv)

                    # ---- gate ----------------------------------------
                    SILU = mybir.ActivationFunctionType.Silu
                    if cfg["fuse_s"]:
                        h0g = h0_pool.tile([128, 2, nf], BF16, tag="h0")
                        nc.scalar.activation(h0g[:], ps_s[:], SILU)
                        h0 = h0g[:, 0, :]
                        g_ap = h0g[:, 1, :]
                    else:
                        h0t = h0_pool.tile([128, nf], BF16, tag="h0")
                        nc.scalar.activation(h0t[:], ps_a[:], SILU)
                        h0 = h0t[:]
                        gt = g_pool.tile([128, nf], FP32, tag="g")
                        nc.scalar.activation(gt[:], ps_b[:], SILU)
                        g_ap = gt[:]
                    if cfg["fuse_v"]:
                        h1t = h1_pool.tile([128, 3, nf], BF16, tag="h1")
                        gb = g_ap.rearrange("p (o n) -> p o n", o=1).broadcast_to(
                            [128, 3, nf]
                        )
                        ENG[cfg["fmul_eng"]].tensor_mul(h1t[:], ps_vt[:], gb)
                        h1 = [h1t[:, i, :] for i in range(3)]
                    else:
                        h1 = []
                        for i in range(3):
                            hi = h1_pool.tile([128, nf], BF16, tag="h1")
                            ENG[cfg["mul_eng"][i]].tensor_mul(hi[:], ps_v[i][:], g_ap)
                            h1.append(hi[:])

                    # ---- mm2: activations stationary -> node-major ----
                    def _copy(eng_key, dst, src_ap):
                        eng = ENG[eng_key]
                        if eng is nc.scalar:
                            eng.copy(dst, src_ap)
                        else:
                            eng.tensor_copy(dst, src_ap)

                    if cfg["mm2_wstat"]:
                        for pidx, (act, w2) in enumerate(
                            [(h0, w2s), (h1[0], w2v), (h1[1], w2v), (h1[2], w2v)]
                        ):
                            py = psY.tile([128, nf], FP32, tag="psY")
                            nc.tensor.matmul(
                                py[:], w2, act[:], start=True, stop=True
                            )
                            dst = yout[:, pidx, sl]
                            _copy(cfg["cpy_eng"][pidx], dst, py[:])
                    elif cfg["fuse_y"]:
                        py0 = psY.tile([128, nf], FP32, tag="psY0")
                        for j in range(S):
                            nc.tensor.matmul(
                                py0[:, j * 128 : (j + 1) * 128],
                                h0[:, j * 128 : (j + 1) * 128],
                                w2s, start=True, stop=True,
                            )
                        _copy(
                            cfg["ycpy0_eng"],
                            yout[:, s0 : s0 + S, 0:128],
                            py0[:].rearrange("p (s n) -> p s n", s=S),
                        )
                        pyv = psY.tile([128, 3, nf], FP32, tag="psYv")
                        for i in range(3):
                            for j in range(S):
                                nc.tensor.matmul(
                                    pyv[:, i, j * 128 : (j + 1) * 128],
                                    h1[i][:, j * 128 : (j + 1) * 128],
                                    w2v, start=True, stop=True,
                                )
                        dst = yout[:, s0 : s0 + S, MUL:].rearrange(
                            "p s (c n) -> p s c n", c=3
                        )
                        src_ap = pyv[:].rearrange("p c (s n) -> p s c n", s=S)
                        _copy(cfg["fcpy_eng"], dst, src_ap)
                    else:
                        for pidx, (act, w2) in enumerate(
                            [(h0, w2s), (h1[0], w2v), (h1[1], w2v), (h1[2], w2v)]
                        ):
                            py = psY.tile([128, nf], FP32, tag="psY")
                            for j in range(S):
                                nc.tensor.matmul(
                                    py[:, j * 128 : (j + 1) * 128],
                                    act[:, j * 128 : (j + 1) * 128],
                                    w2, start=True, stop=True,
                                )
                            dst = yout[:, s0 : s0 + S, pidx * 128 : (pidx + 1) * 128]
                            src_ap = py[:].rearrange("p (s n) -> p s n", s=S)
                            _copy(cfg["cpy_eng"][pidx], dst, src_ap)

                    s0 += S

                # ---- store chunk (optionally in pieces for earlier drain) -
                out_eng = nc.sync if cfg["out_dma_eng"] == "s" else nc.scalar
                if cfg["mm2_wstat"]:
                    if cfg["xblock"]:
                        out_eng.dma_start(y_d[:, ci, :, :], yout[:])
                    else:
                        out_eng.dma_start(y_d[:, :, r0 : r0 + rows], yout[:])
                else:
                    n_out = cfg.get("out_split", 1)
                    tper = (T + n_out - 1) // n_out
                    o0 = 0
                    while o0 < T:
                        ot = min(tper, T - o0)
                        dst = y_d[r0 + o0 * 128 : r0 + (o0 + ot) * 128, :].rearrange(
                            "(t p) f -> p t f", p=128
                        )
                        out_eng.dma_start(dst, yout[:, o0 : o0 + ot, :])
                        o0 += ot

                t0 += T


def build_bass(n_rows=ROWS_PER_CORE, repeats=1, cfg=None):
    full_cfg = {**DEFAULT_CFG, **(cfg or {})}
    nc = bass.Bass(trn_type="TRN2", target_bir_lowering=False, debug=False)
    if full_cfg["xblock"]:
        ct = full_cfg["chunk_t"]
        assert (n_rows // 128) % ct == 0 and not full_cfg["taper"]
        nch = (n_rows // 128) // ct
        x_d = nc.dram_tensor(
            "x", [128, nch, 4, ct * 128], BF16, kind="ExternalInput"
        ).ap()
    else:
        x_d = nc.dram_tensor("x", [128, 4, n_rows], BF16, kind="ExternalInput").ap()
    if full_cfg["packw"]:
        w1s_d = nc.dram_tensor("w", [MUL, 5 * MUL], BF16, kind="ExternalInput").ap()
        w1v_d = w2s_d = w2v_d = None
    else:
        w1s_d = nc.dram_tensor("w1_s", [MUL, 2 * MUL], BF16, kind="ExternalInput").ap()
        w1v_d = nc.dram_tensor("w1_v", [MUL, MUL], BF16, kind="ExternalInput").ap()
        w2s_d = nc.dram_tensor("w2_s", [MUL, MUL], BF16, kind="ExternalInput").ap()
        w2v_d = nc.dram_tensor("w2_v", [MUL, MUL], BF16, kind="ExternalInput").ap()
    if full_cfg["mm2_wstat"]:
        if full_cfg["xblock"]:
            y_d = nc.dram_tensor(
                "y", [128, nch, 4, ct * 128], BF16, kind="ExternalOutput"
            ).ap()
        else:
            y_d = nc.dram_tensor(
                "y", [128, 4, n_rows], BF16, kind="ExternalOutput"
            ).ap()
    else:
        y_d = nc.dram_tensor("y", [n_rows, F], BF16, kind="ExternalOutput").ap()
    with SplitDrainTileContext(nc) as tc:
        build_ir(tc, y_d, x_d, w1s_d, w1v_d, w2s_d, w2v_d, n_rows, repeats=repeats, cfg=cfg)
    return nc


def _pack_x(x_shard, cfg=None):
    """[rows, 512] fp32 -> [128(c), 4(g), rows] bf16, de-interleaved.

    With cfg[xblock]: -> [128, n_chunks, 4, chunk_rows] (chunk-contiguous)."""
    cfg = {**DEFAULT_CFG, **(cfg or {})}
    rows = x_shard.shape[0]
    xb = x_shard.astype(ml_dtypes.bfloat16)
    out = np.empty((128, 4, rows), ml_dtypes.bfloat16)
    out[:, 0, :] = xb[:, :MUL].T
    x1 = xb[:, MUL:].reshape(rows, MUL, 3)
    for i in range(3):
        out[:, 1 + i, :] = x1[:, :, i].T
    if cfg["xblock"]:
        ctr = cfg["chunk_t"] * 128
        nch = rows // ctr
        # [128, 4, nch, ctr] -> [128, nch, 4, ctr]
        out = np.ascontiguousarray(
            out.reshape(128, 4, nch, ctr).transpose(0, 2, 1, 3)
        )
    return out


def shard_inputs(x, w1_s, w1_v, w2_s, w2_v, n_rows=ROWS_PER_CORE, cfg=None):
    x = np.ascontiguousarray(np.asarray(x, dtype=np.float32))
    pad = N_CORES * n_rows - x.shape[0]
    if pad:
        x = np.concatenate([x, np.zeros((pad, x.shape[1]), np.float32)], axis=0)
    shards = x.reshape(N_CORES, n_rows, F)
    full_cfg = {**DEFAULT_CFG, **(cfg or {})}
    if full_cfg["packw"]:
        wcat = np.concatenate(
            [
                np.asarray(w1_s, np.float32),
                np.asarray(w1_v, np.float32),
                np.asarray(w2_s, np.float32),
                np.asarray(w2_v, np.float32),
            ],
            axis=1,
        )
        w = {"w": (wcat * INV).astype(ml_dtypes.bfloat16)}
    else:
        w = {
            "w1_s": (np.asarray(w1_s, np.float32) * INV).astype(ml_dtypes.bfloat16),
            "w1_v": (np.asarray(w1_v, np.float32) * INV).astype(ml_dtypes.bfloat16),
            "w2_s": (np.asarray(w2_s, np.float32) * INV).astype(ml_dtypes.bfloat16),
            "w2_v": (np.asarray(w2_v, np.float32) * INV).astype(ml_dtypes.bfloat16),
        }
    return [dict(w, x=_pack_x(shards[c], cfg=cfg)) for c in range(N_CORES)]


def _unpack_y(y_all, n_out):
    """[n, 512] bf16 col-groups [y0|y1_0|y1_1|y1_2] -> fp32 interleaved."""
    y = y_all[:n_out]
    out = np.empty((n_out, F), np.float32)
    out[:, :MUL] = y[:, :MUL]
    yv = np.asarray(y[:, MUL:], dtype=np.float32).reshape(n_out, 3, MUL)
    out[:, MUL:] = yv.transpose(0, 2, 1).reshape(n_out, 3 * MUL)
    return out


def _unpack_y_wstat(y_cores, n_out):
    """list of [128(v), 4(g), rows] (or chunk-blocked) bf16 -> fp32 interleaved."""
    out_blocks = []
    for yd in y_cores:
        if yd.ndim == 4:  # [128, nch, 4, ctr] -> [128, 4, rows]
            yd = yd.transpose(0, 2, 1, 3).reshape(128, 4, -1)
        rows = yd.shape[2]
        blk = np.empty((rows, F), np.float32)
        blk[:, :MUL] = yd[:, 0, :].T
        yv = np.asarray(yd[:, 1:, :], dtype=np.float32)  # [128v, 3, rows]
        blk[:, MUL:] = yv.transpose(2, 0, 1).reshape(rows, 3 * MUL)
        out_blocks.append(blk)
    return np.concatenate(out_blocks, axis=0)[:n_out]


_NC_CACHE = {}


def kernel(x, w1_s, w1_v, w2_s, w2_v):
    from concourse.bass_utils import run_bass_kernel_spmd

    if "nc" not in _NC_CACHE:
        _NC_CACHE["nc"] = build_bass()
    nc = _NC_CACHE["nc"]
    in_maps = shard_inputs(x, w1_s, w1_v, w2_s, w2_v)
    res = run_bass_kernel_spmd(nc, in_maps, core_ids=list(range(N_CORES)))
    if DEFAULT_CFG["mm2_wstat"]:
        return _unpack_y_wstat([res.results[c]["y"] for c in range(N_CORES)], N_FULL)
    y = np.concatenate([res.results[c]["y"] for c in range(N_CORES)], axis=0)
    return _unpack_y(y, N_FULL)
